# revision 47
# baseline (speedup 1.0000x reference)
"""Trainium2 Bass kernel for AdaptiveScaledDotProductAttention (v3.1).

Sharding: DP=4 over batch x sequence-parallel over query halves
(8 NeuronCores, core = 2*batch + half). Each core runs ALL 16 heads for
its 512 queries; k/v projections over the full 1024 keys are duplicated
within a pair. No collectives: every core produces its own 512 output
rows end-to-end, the host concatenates.

Schedule: k,q projections first; five heads' score matmuls are
interleaved into the v/s projection chunks so the Activation engine
(exp) starts ~40us early and attention stays PE-paced behind a 6-deep
E ring. Softmax tails run per head-pair so the combine overlaps later
pairs' attention. Elementwise adds and q*s products go to the Pool
engine; PSUM evacuation splits between DVE and Act.
"""

import numpy as np

H, DK, DV, DM = 16, 64, 64, 1024
B, N = 4, 1024
SCALE = float(1.0 / np.sqrt(DK))
NCORES = 8
HP = 8          # head pairs per core (all 16 heads)
NQ = N // 2     # queries per core
HDV = H * DV    # 1024
EARLY = 6       # heads whose scores interleave with v/s projections

_CACHE = {}


def _build(with_biases, k_iter=1):
    import concourse.bass as bass  # noqa: F401
    import concourse.tile as tile
    from concourse import bacc, mybir
    from contextlib import ExitStack

    f32 = mybir.dt.float32
    bf16 = mybir.dt.bfloat16
    Exp = mybir.ActivationFunctionType.Exp
    Copy = mybir.ActivationFunctionType.Copy

    nc = bacc.Bacc("TRN2", target_bir_lowering=False, debug=False,
                   num_devices=NCORES)

    def din(name, shape, dt=bf16):
        return nc.dram_tensor(name, shape, dt, kind="ExternalInput").ap()

    # feature-major activations [dm, tokens]; q/s pre-sliced to own half
    xqT = din("xqT", [DM, NQ])
    xkT = din("xkT", [DM, N])
    xvT = din("xvT", [DM, N])
    xsT = din("xsT", [DM, NQ])
    # weights host-prearranged to [p, ht, j, c] (SBUF layout)
    wq = din("wq", [128, 8, 8, 128])
    wk = din("wk", [128, 8, 8, 128])
    wv = din("wv", [128, 8, 8, 128])
    ws = din("ws", [128, 8, 8, 128])
    wo = din("wo", [HDV, DM])  # rows pre-permuted to paired layout
    selc = din("selc", [34, 128])
    if with_biases:
        bq = din("bq", [1, HDV])
        bk = din("bk", [1, HDV])
        bv = din("bv", [1, HDV])
        bs = din("bs", [1, HDV])
        bo = din("bo", [1, DM])
    out = nc.dram_tensor("out", [NQ, DM], bf16, kind="ExternalOutput").ap()

    with ExitStack() as top:
        tc = top.enter_context(tile.TileContext(nc))
        persist = top.enter_context(tc.tile_pool(name="persist", bufs=1))

        # paired feature-major projections: [128 = 2x64 dims, pair, token]
        qT2 = persist.tile([128, HP, NQ], bf16)
        kT2 = persist.tile([128, HP, N], bf16)
        sT2 = persist.tile([128, HP, NQ], bf16)
        # token-major V: [128 keys, 8 chunks, head, 64v|0|0|1]
        # cols 64,65 zero (lang rows), col 66 ones (denominator row)
        vaug = persist.tile([128, 8, H, DV + 3], bf16)
        outT2 = persist.tile([128, HP, NQ], bf16)
        # lang selector columns: col par = ones on parity's 64 rows
        lang2 = persist.tile([128, 2], bf16)
        # broadcast selector: sel2[r, m] = 1 iff r == m//64
        sel2 = persist.tile([34, 128], bf16)

        nc.gpsimd.memset(vaug[:, :, :, DV:DV + 2], 0.0)
        nc.gpsimd.memset(vaug[:, :, :, DV + 2:DV + 3], 1.0)
        nc.gpsimd.memset(lang2[:, :], 0.0)
        nc.gpsimd.memset(lang2[0:64, 0:1], 1.0)
        nc.gpsimd.memset(lang2[64:128, 1:2], 1.0)
        nc.sync.dma_start(out=sel2, in_=selc)
        if with_biases:
            onesrow = persist.tile([1, 512], bf16)
            nc.gpsimd.memset(onesrow[:, :], 1.0)

        for _it in range(k_iter):
            with ExitStack() as body:
                wopool = body.enter_context(
                    tc.tile_pool(name="wopool", bufs=1))
                pb = body.enter_context(
                    tc.tile_pool(name="pb", bufs=2, space="PSUM"))
                scp = body.enter_context(
                    tc.tile_pool(name="scp", bufs=2, space="PSUM"))
                avp = body.enter_context(
                    tc.tile_pool(name="avp", bufs=2, space="PSUM"))
                ep = body.enter_context(
                    tc.tile_pool(name="ep", bufs=EARLY + 1))

                wo_sb = wopool.tile([128, 8, DM], bf16, tag="wo")
                if with_biases:
                    bias_sb = {}
                    for nm, ap in (("bq", bq), ("bk", bk), ("bv", bv),
                                   ("bs", bs), ("bo", bo)):
                        t = wopool.tile([1, ap.shape[1]], bf16, tag=f"b_{nm}")
                        nc.sync.dma_start(out=t, in_=ap)
                        bias_sb[nm] = t

                def scores(l, pool=None):
                    hp, par = l // 2, l % 2
                    pbs = 64 * par
                    E = (pool or ep).tile([128, 8, 512], bf16, tag="E",
                                          name="E")
                    for kc2 in range(4):
                        sc = scp.tile([128, 1024], f32, tag="sc", name="sc")
                        for half in range(2):
                            c = 2 * kc2 + half
                            nc.tensor.matmul(
                                sc[:, half * 512:(half + 1) * 512],
                                kT2[pbs:pbs + 64, hp, c * 128:(c + 1) * 128],
                                qT2[pbs:pbs + 64, hp, :],
                                start=True, stop=True)
                        nc.scalar.activation(
                            E[:, 2 * kc2:2 * kc2 + 2, :].rearrange(
                                "p a b -> p (a b)"),
                            sc, Exp, scale=SCALE)
                    return E

                with ExitStack() as projscope:
                    xpool = projscope.enter_context(
                        tc.tile_pool(name="xpool", bufs=2))
                    wpool = projscope.enter_context(
                        tc.tile_pool(name="wpool", bufs=2))

                    # loads; first k operands split so PE starts early
                    w_sb, x_sb = {}, {}

                    for i, (wnm, wap, xnm, xap, xn) in enumerate((
                            ("wq", wq, "xq", xqT, NQ),
                            ("wk", wk, "xk", xkT, N),
                            ("wv", wv, "xv", xvT, N),
                            ("ws", ws, "xs", xsT, NQ))):
                        xsrc = xap.rearrange("(j p) t -> p j t", p=128)
                        tw = wpool.tile([128, 8, 8, 128], bf16, tag="w",
                                        name=wnm, bufs=2)
                        # first column-chunk lands fast so the PE starts
                        # early; remaining chunks stream behind it
                        nc.sync.dma_start(out=tw[:, 0, :, :],
                                          in_=wap[:, 0, :, :])
                        txs = []
                        for ii in range(4):
                            js = slice(ii * 2, (ii + 1) * 2)
                            tx = xpool.tile([128, 2, xn], bf16, tag="x",
                                            name=f"{xnm}{ii}", bufs=8)
                            nc.sync.dma_start(out=tx, in_=xsrc[:, js, :])
                            txs.append(tx)
                        for ht in range(1, 8):
                            nc.sync.dma_start(out=tw[:, ht, :, :],
                                              in_=wap[:, ht, :, :])
                        w_sb[wnm] = tw
                        x_sb[xnm] = txs
                    nc.sync.dma_start(
                        out=wo_sb, in_=wo.rearrange("(c p) m -> p c m", p=128))

                    def proj_chunk(wnm, xnm, bnm, dst, ht, ts_):
                        ps = pb.tile([128, 512], f32, tag="ps", name="ps")
                        for j in range(8):
                            nc.tensor.matmul(
                                ps,
                                w_sb[wnm][:, ht, j, :],
                                x_sb[xnm][j // 2][:, j % 2, ts_],
                                start=(j == 0),
                                stop=(j == 7 and not with_biases))
                        if with_biases:
                            nc.tensor.matmul(
                                ps,
                                bias_sb[bnm][:, ht * 128:(ht + 1) * 128],
                                onesrow[:, :],
                                start=False, stop=True)
                        nc.vector.tensor_copy(out=dst[:, ht, ts_], in_=ps)

                    def vproj_chunk(kc, hh):
                        hs_ = slice(hh * 512, (hh + 1) * 512)
                        ps = pb.tile([128, 512], f32, tag="ps", name="ps")
                        for j in range(8):
                            nc.tensor.matmul(
                                ps,
                                x_sb["xv"][j // 2][
                                    :, j % 2, kc * 128:(kc + 1) * 128],
                                w_sb["wv"][:, 4 * hh:4 * hh + 4, j, :],
                                start=(j == 0),
                                stop=(j == 7 and not with_biases))
                        if with_biases:
                            nc.tensor.matmul(
                                ps, onesrow[:, 0:128], bias_sb["bv"][:, hs_],
                                start=False, stop=True)
                        nc.vector.tensor_copy(
                            out=vaug[:, kc, hh * 8:(hh + 1) * 8, 0:DV],
                            in_=ps.rearrange("p (h d) -> p h d", h=8))

                    for ht in range(HP):
                        proj_chunk("wq", "xq", "bq", qT2, ht, slice(0, NQ))
                    Es = {}
                    nxt = 0
                    for ht in range(HP):
                        for nchi in range(2):
                            proj_chunk("wk", "xk", "bk", kT2, ht,
                                       slice(nchi * 512, (nchi + 1) * 512))
                        if ht in (1, 3, 5):
                            Es[nxt] = scores(nxt)
                            nxt += 1
                            Es[nxt] = scores(nxt)
                            nxt += 1

                    # v and s projections with early scores interleaved
                    chunks = [("v", kc, hh) for kc in range(8)
                              for hh in range(2)]
                    chunks += [("s", ht, None) for ht in range(HP)]
                    for i, (kind, a, b_) in enumerate(chunks):
                        if kind == "v":
                            vproj_chunk(a, b_)
                        else:
                            proj_chunk("ws", "xs", "bs", sT2, a, slice(0, NQ))
                        if i % 4 == 2 and nxt < EARLY:
                            Es[nxt] = scores(nxt)
                            nxt += 1

                # ---- attention pools (proj x/w SBUF now free) ----
                u2p = body.enter_context(tc.tile_pool(name="u2p", bufs=1))
                p2p = body.enter_context(tc.tile_pool(name="p2p", bufs=2))
                tp = body.enter_context(tc.tile_pool(name="tp", bufs=2))
                osp = body.enter_context(tc.tile_pool(name="osp", bufs=8))

                u2 = u2p.tile([128, HP, 512], f32, tag="u2")
                oph = {}

                def attend(l, E, p2t, prAB, els):
                    hp, par = l // 2, l % 2
                    av = avp.tile([128, 512], f32, tag="av", name="av")
                    for c in range(8):
                        nc.tensor.matmul(
                            av[0:DV + 3, :], vaug[:, c, l, :], E[:, c, :],
                            start=(c == 0), stop=(c == 7 and par == 1))
                    if par == 0:
                        # both heads' lang logits into rows 64:66
                        nc.tensor.matmul(
                            av[DV:DV + 2, :], lang2, p2t,
                            start=False, stop=True)
                        # exp straight from PSUM, concurrent with den DMA
                        nc.scalar.activation(els[32:34, :], av[64:66, :],
                                             Exp, scale=SCALE)
                        nc.vector.tensor_copy(
                            out=u2[0:64, hp, :], in_=av[0:DV, :])
                    else:
                        nc.vector.tensor_copy(
                            out=u2[64:128, hp, :], in_=av[0:DV, :])
                    st2 = tp.tile([3, 512], f32, tag="st2", name="st2")
                    nc.vector.tensor_copy(out=st2, in_=av[64:67, :])
                    nc.sync.dma_start(
                        out=prAB[32 + par:33 + par, :], in_=st2[2:3, :])

                def pair_tail(hp, prAB, el):
                    # whole chain lives at partition base 32 so SB+SB
                    # tensor_tensor inputs share their start partition
                    dn = tp.tile([34, 512], f32, tag="dn", name="dn")
                    nc.vector.tensor_add(dn[32:34, :], prAB[32:34, :],
                                         el[32:34, :])
                    rcb = tp.tile([34, 512], bf16, tag="rcb", name="rcb")
                    with nc.allow_low_precision(reason="1/denom in bf16"):
                        nc.vector.reciprocal(rcb[32:34, :], dn[32:34, :])
                    w2b = tp.tile([34, 512], bf16, tag="w2b", name="w2b")
                    nc.vector.tensor_mul(w2b[32:34, :], rcb[32:34, :],
                                         el[32:34, :])
                    rc2 = pb.tile([128, 512], f32, tag="ps", name="rc2")
                    nc.tensor.matmul(rc2, sel2[32:34, :], rcb[32:34, :],
                                     start=True, stop=True)
                    w22 = pb.tile([128, 512], f32, tag="ps", name="w22")
                    nc.tensor.matmul(w22, sel2[32:34, :], w2b[32:34, :],
                                     start=True, stop=True)
                    t1 = tp.tile([128, 512], f32, tag="t1", name="t1")
                    nc.vector.tensor_mul(t1, u2[:, hp, :], rc2)
                    t2 = tp.tile([128, 512], f32, tag="t2", name="t2")
                    nc.vector.tensor_mul(t2, sT2[:, hp, :], w22)
                    if hp == HP - 1:
                        nc.vector.tensor_add(outT2[:, hp, :], t1, t2)
                    else:
                        nc.gpsimd.tensor_add(outT2[:, hp, :], t1, t2)

                prAB = None
                for l in range(H):
                    hp, par = l // 2, l % 2
                    if par == 0:
                        p2t = p2p.tile([128, 512], bf16, tag="p2", name="p2")
                        nc.gpsimd.tensor_mul(
                            p2t, qT2[:, hp, :], sT2[:, hp, :])
                        prAB = tp.tile([34, 512], f32, tag="prAB",
                                       name="prAB")
                        els = tp.tile([34, 512], f32, tag="el", name="el")
                    if l not in Es:
                        Es[l] = scores(l)
                    attend(l, Es.pop(l), p2t, prAB, els)
                    if l + EARLY + 1 < H:
                        Es[l + EARLY + 1] = scores(l + EARLY + 1)
                    if par == 1:
                        pair_tail(hp, prAB, els)
                    if l in (9, 11, 13):
                        seg = (0, 4)
                        third = {9: 0, 11: 1, 13: 2}[l]
                        for nt in range(4):
                            for dh in range(2):
                                if min((2 * nt + dh) // 3, 2) != third:
                                    continue
                                ds = slice(dh * 512, (dh + 1) * 512)
                                ps = pb.tile([128, 512], f32, tag="ps",
                                             name="oph")
                                for c in range(*seg):
                                    nc.tensor.matmul(
                                        ps,
                                        outT2[:, c, nt * 128:(nt + 1) * 128],
                                        wo_sb[:, c, ds],
                                        start=(c == seg[0]),
                                        stop=(c == seg[1] - 1))
                                t = osp.tile([128, 512], bf16, tag="oph",
                                             name="ophs")
                                if (nt + dh) % 2 == 0:
                                    nc.vector.tensor_copy(out=t, in_=ps)
                                else:
                                    nc.scalar.activation(t, ps, Copy)
                                oph[2 * nt + dh] = t

                # ---- output projection second half + combine ----
                for nt in range(4):
                    for dh in range(2):
                        ds = slice(dh * 512, (dh + 1) * 512)
                        ps = pb.tile([128, 512], f32, tag="ps", name="ops")
                        for c in range(4, 8):
                            nc.tensor.matmul(
                                ps,
                                outT2[:, c, nt * 128:(nt + 1) * 128],
                                wo_sb[:, c, ds],
                                start=(c == 4),
                                stop=(c == 7 and not with_biases))
                        if with_biases:
                            nc.tensor.matmul(
                                ps, onesrow[:, 0:128], bias_sb["bo"][:, ds],
                                start=False, stop=True)
                        st = osp.tile([128, 512], bf16, tag="ost", name="ost", bufs=4)
                        nc.vector.tensor_add(st, oph[2 * nt + dh], ps)
                        nc.sync.dma_start(
                            out=out[nt * 128:(nt + 1) * 128, ds], in_=st)

    nc.compile()
    return nc


def _get_nc(with_biases, k_iter=1):
    key = ("nc", with_biases, k_iter)
    if key not in _CACHE:
        _CACHE[key] = _build(with_biases, k_iter)
    return _CACHE[key]


def _wo_perm():
    # on-chip outT2 row r (= chunk c*128 + partition p) -> original Wo row
    r = np.arange(HDV)
    c = r // 128
    p = r % 128
    head = 2 * c + (p // 64)
    return head * 64 + (p % 64)


def _w_pre(W):
    # [1024, 1024] -> [p, ht, j, c] matching the SBUF stationary layout
    import ml_dtypes
    return np.ascontiguousarray(
        np.asarray(W).reshape(8, 128, 8, 128).transpose(1, 2, 0, 3)
    ).astype(ml_dtypes.bfloat16)


def _sel_mat():
    import ml_dtypes
    sel = np.zeros((34, 128), np.float32)
    sel[32, 0:64] = 1.0
    sel[33, 64:128] = 1.0
    return sel.astype(ml_dtypes.bfloat16)


def _in_maps(queries, keys, values, language_signals,
             Wq, b_q, Wk, b_k, Wv, b_v, Ws, b_s, Wo, b_o, with_biases):
    import ml_dtypes
    bf = ml_dtypes.bfloat16
    perm = _wo_perm()
    selm = _sel_mat()
    in_maps = []
    for core in range(NCORES):
        b, h2 = core // 2, core % 2
        qs = slice(NQ * h2, NQ * (h2 + 1))
        m = {
            "xqT": np.ascontiguousarray(queries[b, qs].T).astype(bf),
            "xkT": np.ascontiguousarray(keys[b].T).astype(bf),
            "xvT": np.ascontiguousarray(values[b].T).astype(bf),
            "xsT": np.ascontiguousarray(language_signals[b, qs].T).astype(bf),
            "wq": _w_pre(Wq),
            "wk": _w_pre(Wk),
            "wv": _w_pre(Wv),
            "ws": _w_pre(Ws),
            "wo": np.ascontiguousarray(Wo[perm]).astype(bf),
            "selc": selm,
        }
        if with_biases:
            m["bq"] = np.asarray(b_q, np.float32).reshape(1, -1).astype(bf)
            m["bk"] = np.asarray(b_k, np.float32).reshape(1, -1).astype(bf)
            m["bv"] = np.asarray(b_v, np.float32).reshape(1, -1).astype(bf)
            m["bs"] = np.asarray(b_s, np.float32).reshape(1, -1).astype(bf)
            m["bo"] = np.asarray(b_o, np.float32).reshape(1, -1).astype(bf)
        in_maps.append(m)
    return in_maps


def kernel(queries, keys, values, language_signals,
           Wq, b_q, Wk, b_k, Wv, b_v, Ws, b_s, Wo, b_o):
    from concourse.bass_utils import run_bass_kernel_spmd

    with_biases = any(
        np.any(np.asarray(b)) for b in (b_q, b_k, b_v, b_s, b_o))
    nc = _get_nc(with_biases)
    in_maps = _in_maps(queries, keys, values, language_signals,
                       Wq, b_q, Wk, b_k, Wv, b_v, Ws, b_s, Wo, b_o,
                       with_biases)
    _CACHE["last_in_maps"] = in_maps
    _CACHE["last_with_biases"] = with_biases
    res = run_bass_kernel_spmd(nc, in_maps, list(range(NCORES))).results
    full = np.empty((B, N, DM), np.float32)
    for core in range(NCORES):
        b, h2 = core // 2, core % 2
        full[b, NQ * h2:NQ * (h2 + 1), :] = \
            np.asarray(res[core]["out"]).astype(np.float32)
    return full
